# revision 3
# baseline (speedup 1.0000x reference)
"""ClassAttention kernel for 8x TRN2 NeuronCores (Bass/Tile).

Problem (hardcoded): x[16, 2049, 1024], w_qkv[3072, 1024], w_proj[1024, 1024],
b_proj[1024].  Reference computes qkv projection, class-token attention
(only query position 0 attends), projection of the class token, and returns
concat([cls_tok, x[:, 1:]], axis=1).

Only output row 0 is computed; rows 1.. are x passthrough (done on host at
gather time, mirroring the reference's concatenate).

Algebraic restructure (exact same math, far fewer FLOPs):
    q0[b]        = x[b,0] @ Wq^T                       (host, tiny)
    wfold[b,h,:] = SCALE * q0[b,h,:] @ Wk_h            (host, tiny: fold q0 into Wk)
    logits[b,h,s]= sum_d x[b,s,d] * wfold[b,h,d]       (device matmul over d)
    attn         = softmax_s(logits)                    (device)
    xaT[b,d,h]   = sum_s x[b,s,d] * attn[b,h,s]        (device matmul over s)
    cls[b,he]    = sum_d xaT[b,d,h] * WvT[d,he]        (device, per-head blocks)
    out0[b,f]    = sum_d cls[b,d] * WpT[d,f] + bp[f]   (device)

Sharding: data-parallel over batch, 2 batch elements per core (8 cores).
x is shipped in bf16 in both natural [s,d] and transposed [d,s] layouts so
both contractions stream from SBUF with the contraction on the partition dim.
"""

import os
import numpy as np
import ml_dtypes

BF16 = ml_dtypes.bfloat16

B, S, D, H, E = 16, 2049, 1024, 16, 64
SCALE = E ** -0.5
NCORES = 8
BL = B // NCORES          # batches per core = 2
ST = 17                   # s-tiles of 128 (padded)
SP = ST * 128             # 2176 padded sequence
DT = 8                    # d-tiles of 128
NEG_BIG = -30000.0        # exp() underflows to exactly 0 in fp32

_cached = {}


def _kernel_body(ctx, tc):
    import concourse.bass as bass
    from concourse import mybir

    nc = tc.nc
    dt = mybir.dt
    AF = mybir.ActivationFunctionType

    xn_d = nc.dram_tensor("xn", (BL * SP, D), dt.bfloat16, kind="ExternalInput").ap()
    xt_d = nc.dram_tensor("xt", (BL * D, S), dt.bfloat16, kind="ExternalInput").ap()
    wf_d = nc.dram_tensor("wf", (128, BL * 128), dt.bfloat16, kind="ExternalInput").ap()
    wv_d = nc.dram_tensor("wv", (D, D), dt.bfloat16, kind="ExternalInput").ap()
    wp_d = nc.dram_tensor("wp", (D, D), dt.bfloat16, kind="ExternalInput").ap()
    bp_d = nc.dram_tensor("bp", (128, 8), dt.float32, kind="ExternalInput").ap()
    id_d = nc.dram_tensor("ident", (16, 16), dt.bfloat16, kind="ExternalInput").ap()
    out_d = nc.dram_tensor("out", (128, 16), dt.float32, kind="ExternalOutput").ap()

    cpool = ctx.enter_context(tc.tile_pool(name="const", bufs=1))
    xn_pool = ctx.enter_context(tc.tile_pool(name="xn", bufs=1))
    xt_pool = ctx.enter_context(tc.tile_pool(name="xt", bufs=3))
    w_pool = ctx.enter_context(tc.tile_pool(name="w", bufs=1))
    sm_pool = ctx.enter_context(tc.tile_pool(name="sm", bufs=1))
    st_pool = ctx.enter_context(tc.tile_pool(name="stats", bufs=2))
    at_pool = ctx.enter_context(tc.tile_pool(name="attnT", bufs=2))
    acc_pool = ctx.enter_context(tc.tile_pool(name="acc", bufs=1))

    ps_log = ctx.enter_context(tc.tile_pool(name="pslog", bufs=1, space="PSUM"))
    ps_tr = ctx.enter_context(tc.tile_pool(name="pstr", bufs=2, space="PSUM"))
    ps_xa = ctx.enter_context(tc.tile_pool(name="psxa", bufs=1, space="PSUM"))
    ps_sm = ps_tr

    # --- constants / weights ---
    wf_sb = cpool.tile([128, BL * 128], dt.bfloat16, tag="wf")
    nc.sync.dma_start(wf_sb[:], wf_d)
    id_sb = cpool.tile([16, 16], dt.bfloat16, tag="ident")
    nc.sync.dma_start(id_sb[:], id_d)
    bp_sb = cpool.tile([128, 8], dt.float32, tag="bp")
    nc.sync.dma_start(bp_sb[:], bp_d)

    # xn: one big persistent tile per batch, [128, st*1024], row (st p) -> p, (st d)
    xn_sb = []
    for b in range(BL):
        t = xn_pool.tile([128, ST * 1024], dt.bfloat16, tag=f"xn{b}")
        src = xn_d[b * SP:(b + 1) * SP, :].rearrange("(st p) d -> p st d", p=128)
        nc.scalar.dma_start(t[:].rearrange("p (st d) -> p st d", st=ST), src)
        xn_sb.append(t)

    # wv / wp: persistent, [128, dt*1024]; slice [:, dtile*1024 + col]
    wv_sb = w_pool.tile([128, DT * 1024], dt.bfloat16, tag="wv")
    nc.scalar.dma_start(
        wv_sb[:].rearrange("p (k c) -> p k c", k=DT),
        wv_d.rearrange("(k p) c -> p k c", p=128),
    )
    wp_sb = w_pool.tile([128, DT * 1024], dt.bfloat16, tag="wp")
    nc.scalar.dma_start(
        wp_sb[:].rearrange("p (k c) -> p k c", k=DT),
        wp_d.rearrange("(k p) c -> p k c", p=128),
    )

    # persistent accumulators
    xaT_sb = acc_pool.tile([128, DT * 2 * H], dt.bfloat16, tag="xaT")  # col=dt*32+h*2+b
    cls_sb = acc_pool.tile([128, DT * BL], dt.bfloat16, tag="clsT")    # col=dtp*2+b
    out_sb = acc_pool.tile([128, 16], dt.float32, tag="out")           # col=fb*2+b

    attnT = []
    for b in range(BL):
        # --- load xt in two halves of 4 d-tiles each ---
        halves = []
        for hh in range(2):
            t = xt_pool.tile([128, 4 * S], dt.bfloat16, tag="xt")
            r0 = b * D + hh * 512
            src = xt_d[r0:r0 + 512, :].rearrange("(k p) s -> p k s", p=128)
            nc.sync.dma_start(t[:].rearrange("p (k s) -> p k s", k=4), src)
            halves.append(t)

        # --- logits: out[h, s] accumulated over 8 d-tiles ---
        chunks = [ps_log.tile([16, 512], dt.float32, tag=f"c{sc}", name=f"c{sc}_{b}")
                  for sc in range(5)]
        for d8 in range(8):
            xtt = halves[d8 // 4]
            lhs = wf_sb[:, b * 128 + d8 * 16: b * 128 + (d8 + 1) * 16]
            base = (d8 % 4) * S
            for sc in range(5):
                n = 512 if sc < 4 else 1
                nc.tensor.matmul(
                    chunks[sc][:, :n], lhs, xtt[:, base + sc * 512: base + sc * 512 + n],
                    start=(d8 == 0), stop=(d8 == 7),
                )

        # --- softmax over s (free dim), padded region = NEG_BIG -> exp = 0 ---
        logits = sm_pool.tile([16, SP], dt.float32, tag="logits")
        nc.vector.memset(logits[:, S:], NEG_BIG)
        for sc in range(5):
            n = 512 if sc < 4 else 1
            nc.vector.tensor_copy(logits[:, sc * 512: sc * 512 + n], chunks[sc][:, :n])
        negmax = st_pool.tile([16, 1], dt.float32, tag="negmax")
        nc.vector.tensor_reduce(
            negmax[:], logits[:], axis=mybir.AxisListType.X,
            op=mybir.AluOpType.max, negate=True,
        )
        expv = sm_pool.tile([16, SP], dt.float32, tag="exp")
        sumexp = st_pool.tile([16, 1], dt.float32, tag="sumexp")
        nc.scalar.activation(expv[:], logits[:], AF.Exp, bias=negmax[:], scale=1.0,
                             accum_out=sumexp[:])
        recip = st_pool.tile([16, 1], dt.float32, tag="recip")
        nc.vector.reciprocal(recip[:], sumexp[:])
        attn = sm_pool.tile([16, SP], dt.bfloat16, tag="attn")
        nc.vector.tensor_scalar_mul(attn[:], expv[:], recip[:])

        # --- transpose attn -> attnT [128s, 16h] per s-tile ---
        atT = at_pool.tile([128, ST * 16], dt.bfloat16, tag="attnT")
        for st in range(ST):
            ps = ps_tr.tile([128, 16], dt.bfloat16, tag="tr")
            nc.tensor.transpose(ps[:], attn[:, st * 128:(st + 1) * 128], id_sb[:])
            nc.vector.tensor_copy(atT[:, st * 16:(st + 1) * 16], ps[:])
        attnT.append(atT)

        # --- xaT[d, h] = sum_s x[s, d] * attn[h, s] ---
        for d8 in range(8):
            ps = ps_xa.tile([128, 16], dt.float32, tag="xa")
            for st in range(ST):
                nc.tensor.matmul(
                    ps[:],
                    xn_sb[b][:, st * 1024 + d8 * 128: st * 1024 + (d8 + 1) * 128],
                    atT[:, st * 16:(st + 1) * 16],
                    start=(st == 0), stop=(st == ST - 1),
                )
            nc.vector.tensor_copy(
                xaT_sb[:, d8 * 32 + b: d8 * 32 + 32: 2], ps[:])

    # --- cls: per head-pair into partition halves ---
    for dp in range(8):
        ps = ps_sm.tile([128, BL], dt.float32, tag="tr", name=f"cls{dp}")
        for half, h in ((0, 2 * dp), (1, 2 * dp + 1)):
            outp = ps[:64, :] if half == 0 else ps[64:128, :]
            for d8 in range(8):
                nc.tensor.matmul(
                    outp,
                    wv_sb[:, d8 * 1024 + h * 64: d8 * 1024 + (h + 1) * 64],
                    xaT_sb[:, d8 * 32 + 2 * h: d8 * 32 + 2 * h + 2],
                    start=(d8 == 0), stop=(d8 == 7),
                    tile_position=(0, 64 * half),
                )
        nc.vector.tensor_copy(cls_sb[:, dp * 2: dp * 2 + 2], ps[:])

    # --- proj + bias ---
    for fb in range(8):
        ps = ps_sm.tile([128, BL], dt.float32, tag="tr", name=f"proj{fb}")
        for dp in range(8):
            nc.tensor.matmul(
                ps[:],
                wp_sb[:, dp * 1024 + fb * 128: dp * 1024 + (fb + 1) * 128],
                cls_sb[:, dp * 2: dp * 2 + 2],
                start=(dp == 0), stop=(dp == 7),
            )
        nc.vector.tensor_scalar_add(out_sb[:, fb * 2: fb * 2 + 2], ps[:],
                                    bp_sb[:, fb: fb + 1])

    nc.sync.dma_start(out_d, out_sb[:])


def _build():
    if "nc" in _cached:
        return _cached["nc"]
    from contextlib import ExitStack
    import concourse.tile as tile
    from concourse import bacc

    nc = bacc.Bacc("TRN2", target_bir_lowering=False, debug=False,
                   num_devices=NCORES)
    with tile.TileContext(nc) as tc:
        with ExitStack() as ctx:
            _kernel_body(ctx, tc)
    nc.compile()
    _cached["nc"] = nc
    return nc


def _host_prep(x, w_qkv, w_proj, b_proj):
    x = np.asarray(x, dtype=np.float32)
    w_qkv = np.asarray(w_qkv, dtype=np.float32)
    w_proj = np.asarray(w_proj, dtype=np.float32)
    b_proj = np.asarray(b_proj, dtype=np.float32)

    w_q, w_k = w_qkv[:D], w_qkv[D:2 * D]
    q0 = x[:, 0, :] @ w_q.T                                   # [B, D]
    wfold = np.einsum("bhe,hed->bhd", q0.reshape(B, H, E),
                      w_k.reshape(H, E, D)) * SCALE           # [B, H, D]
    wfT = np.ascontiguousarray(wfold.transpose(0, 2, 1))      # [B, D, H]

    xb = x.astype(BF16)                                       # [B, S, D]

    wv_dev = np.ascontiguousarray(w_qkv[2 * D:].T).astype(BF16)   # [d, he]
    wp_dev = np.ascontiguousarray(w_proj.T).astype(BF16)          # [d, f]
    bp_dev = np.ascontiguousarray(b_proj.reshape(8, 128).T)       # [p, fb]
    id_dev = np.eye(16, dtype=BF16)

    in_maps = []
    for c in range(NCORES):
        b0 = c * BL
        xs = xb[b0:b0 + BL]                                   # [BL, S, D]
        xn = np.zeros((BL, SP, D), dtype=BF16)
        xn[:, :S] = xs
        xt = np.ascontiguousarray(xs.transpose(0, 2, 1))      # [BL, D, S]
        wf_core = (wfT[b0:b0 + BL].reshape(BL, DT, 128, H)
                   .transpose(2, 0, 1, 3).reshape(128, BL * 128).astype(BF16))
        in_maps.append({
            "xn": xn.reshape(BL * SP, D),
            "xt": xt.reshape(BL * D, S),
            "wf": np.ascontiguousarray(wf_core),
            "wv": wv_dev,
            "wp": wp_dev,
            "bp": bp_dev,
            "ident": id_dev,
        })
    return x, in_maps


def _run(x, w_qkv, w_proj, b_proj, trace=False):
    from concourse import bass_utils

    nc = _build()
    x, in_maps = _host_prep(x, w_qkv, w_proj, b_proj)
    res = bass_utils.run_bass_kernel_spmd(
        nc, in_maps, core_ids=list(range(NCORES)), trace=trace)

    out = x.copy()
    for c in range(NCORES):
        dev = np.asarray(res.results[c]["out"], dtype=np.float32)  # [128, 16]
        cls = dev.reshape(128, 8, BL).transpose(2, 1, 0).reshape(BL, D)
        out[c * BL:(c + 1) * BL, 0, :] = cls
    return out, res


def kernel(x, w_qkv, w_proj, b_proj):
    out, _ = _run(x, w_qkv, w_proj, b_proj, trace=False)
    return out


# revision 4
# speedup vs baseline: 1.0240x; 1.0240x over previous
"""ClassAttention kernel for 8x TRN2 NeuronCores (Bass/Tile).

Problem (hardcoded): x[16, 2049, 1024], w_qkv[3072, 1024], w_proj[1024, 1024],
b_proj[1024].  Reference computes qkv projection, class-token attention
(only query position 0 attends), projection of the class token, and returns
concat([cls_tok, x[:, 1:]], axis=1).

Only output row 0 is computed; rows 1.. are x passthrough (done on host at
gather time, mirroring the reference's concatenate).

Algebraic restructure (exact same math, far fewer FLOPs):
    q0[b]        = x[b,0] @ Wq^T                       (host, tiny)
    wfold[b,h,:] = SCALE * q0[b,h,:] @ Wk_h            (host, tiny: fold q0 into Wk)
    logits[b,h,s]= sum_d x[b,s,d] * wfold[b,h,d]       (device matmul over d)
    attn         = softmax_s(logits)                    (device)
    xaT[b,d,h]   = sum_s x[b,s,d] * attn[b,h,s]        (device matmul over s)
    cls[b,he]    = sum_d xaT[b,d,h] * WvT[d,he]        (device, per-head blocks)
    out0[b,f]    = sum_d cls[b,d] * WpT[d,f] + bp[f]   (device)

Sharding: data-parallel over batch, 2 batch elements per core (8 cores).
x is shipped in bf16 in both natural [s,d] and transposed [d,s] layouts so
both contractions stream from SBUF with the contraction on the partition dim.
"""

import os
import numpy as np
import ml_dtypes

BF16 = ml_dtypes.bfloat16

B, S, D, H, E = 16, 2049, 1024, 16, 64
SCALE = E ** -0.5
NCORES = 8
BL = B // NCORES          # batches per core = 2
ST = 17                   # s-tiles of 128 (padded)
SP = ST * 128             # 2176 padded sequence
DT = 8                    # d-tiles of 128
NEG_BIG = -30000.0        # exp() underflows to exactly 0 in fp32

_cached = {}


def _kernel_body(ctx, tc):
    import concourse.bass as bass
    from concourse import mybir

    nc = tc.nc
    dt = mybir.dt
    AF = mybir.ActivationFunctionType

    xn_d = nc.dram_tensor("xn", (BL * SP, D), dt.bfloat16, kind="ExternalInput").ap()
    xt_d = nc.dram_tensor("xt", (BL * D, S), dt.bfloat16, kind="ExternalInput").ap()
    wf_d = nc.dram_tensor("wf", (128, BL * 128), dt.bfloat16, kind="ExternalInput").ap()
    wv_d = nc.dram_tensor("wv", (D, D), dt.bfloat16, kind="ExternalInput").ap()
    wp_d = nc.dram_tensor("wp", (D, D), dt.bfloat16, kind="ExternalInput").ap()
    bp_d = nc.dram_tensor("bp", (128, 8), dt.float32, kind="ExternalInput").ap()
    id_d = nc.dram_tensor("ident", (16, 16), dt.bfloat16, kind="ExternalInput").ap()
    out_d = nc.dram_tensor("out", (128, 16), dt.float32, kind="ExternalOutput").ap()

    cpool = ctx.enter_context(tc.tile_pool(name="const", bufs=1))
    xn_pool = ctx.enter_context(tc.tile_pool(name="xn", bufs=1))
    xt_pool = ctx.enter_context(tc.tile_pool(name="xt", bufs=3))
    w_pool = ctx.enter_context(tc.tile_pool(name="w", bufs=1))
    sm_pool = ctx.enter_context(tc.tile_pool(name="sm", bufs=1))
    st_pool = ctx.enter_context(tc.tile_pool(name="stats", bufs=2))
    at_pool = ctx.enter_context(tc.tile_pool(name="attnT", bufs=2))
    acc_pool = ctx.enter_context(tc.tile_pool(name="acc", bufs=1))

    ps_log = ctx.enter_context(tc.tile_pool(name="pslog", bufs=1, space="PSUM"))
    ps_tr = ctx.enter_context(tc.tile_pool(name="pstr", bufs=2, space="PSUM"))
    ps_xa = ctx.enter_context(tc.tile_pool(name="psxa", bufs=1, space="PSUM"))
    ps_sm = ps_tr

    # --- constants / weights ---
    wf_sb = cpool.tile([128, BL * 128], dt.bfloat16, tag="wf")
    nc.sync.dma_start(wf_sb[:], wf_d)
    id_sb = cpool.tile([16, 16], dt.bfloat16, tag="ident")
    nc.sync.dma_start(id_sb[:], id_d)
    bp_sb = cpool.tile([128, 8], dt.float32, tag="bp")
    nc.sync.dma_start(bp_sb[:], bp_d)

    # persistent x tiles (natural layout) + weights; all DMA on the sync
    # HWDGE queue in consumption-priority order (FIFO per queue):
    #   consts, xt_b0, xt_b1(h0), xn_b0, xt_b1(h1), wv, xn_b1, wp
    # so early compute is never starved and the last arrival (wp) has the
    # shortest downstream chain (proj only).
    xn_sb = [xn_pool.tile([128, ST * 1024], dt.bfloat16, tag=f"xn{b}",
                          name=f"xn{b}")
             for b in range(BL)]
    wv_sb = w_pool.tile([128, DT * 1024], dt.bfloat16, tag="wv")
    wp_sb = w_pool.tile([128, DT * 1024], dt.bfloat16, tag="wp")

    def load_xn(b, st0, st1):
        src = xn_d[b * SP + st0 * 128: b * SP + st1 * 128, :]
        nc.sync.dma_start(
            xn_sb[b][:, st0 * 1024: st1 * 1024]
            .rearrange("p (st d) -> p st d", st=st1 - st0),
            src.rearrange("(st p) d -> p st d", p=128),
        )

    def load_w(t, src):
        nc.sync.dma_start(
            t[:].rearrange("p (k c) -> p k c", k=DT),
            src.rearrange("(k p) c -> p k c", p=128),
        )

    def load_xt(b, hh):
        t = xt_pool.tile([128, 4 * S], dt.bfloat16, tag="xt", name=f"xt{b}_{hh}")
        r0 = b * D + hh * 512
        src = xt_d[r0:r0 + 512, :].rearrange("(k p) s -> p k s", p=128)
        nc.sync.dma_start(t[:].rearrange("p (k s) -> p k s", k=4), src)
        return t

    # persistent accumulators
    xaT_sb = acc_pool.tile([128, DT * 2 * H], dt.bfloat16, tag="xaT")  # col=dt*32+h*2+b
    cls_sb = acc_pool.tile([128, DT * BL], dt.bfloat16, tag="clsT")    # col=dtp*2+b
    out_sb = acc_pool.tile([128, 16], dt.float32, tag="out")           # col=fb*2+b

    # -- DMA program order (= sync-queue FIFO order) --
    xt_tiles = {}
    xt_tiles[(0, 0)] = load_xt(0, 0)
    xt_tiles[(0, 1)] = load_xt(0, 1)
    xt_tiles[(1, 0)] = load_xt(1, 0)
    load_xn(0, 0, 6)
    load_xn(0, 6, 12)
    load_xn(0, 12, 17)
    xt_tiles[(1, 1)] = load_xt(1, 1)
    load_w(wv_sb, wv_d)
    load_xn(1, 0, 6)
    load_xn(1, 6, 12)
    load_xn(1, 12, 17)
    load_w(wp_sb, wp_d)

    attnT = []
    for b in range(BL):
        halves = [xt_tiles[(b, 0)], xt_tiles[(b, 1)]]

        # --- logits: out[h, s] accumulated over 8 d-tiles, 5 s-chunks in PSUM ---
        chunks = [ps_log.tile([16, 512], dt.float32, tag=f"c{sc}", name=f"c{sc}_{b}")
                  for sc in range(5)]
        for d8 in range(8):
            xtt = halves[d8 // 4]
            lhs = wf_sb[:, b * 128 + d8 * 16: b * 128 + (d8 + 1) * 16]
            base = (d8 % 4) * S
            for sc in range(5):
                n = 512 if sc < 4 else 1
                nc.tensor.matmul(
                    chunks[sc][:, :n], lhs, xtt[:, base + sc * 512: base + sc * 512 + n],
                    start=(d8 == 0), stop=(d8 == 7),
                )

        # --- softmax straight out of PSUM (per-chunk max/exp/sum) ---
        maxes = st_pool.tile([16, 5], dt.float32, tag="maxes")
        for sc in range(5):
            n = 512 if sc < 4 else 1
            nc.vector.tensor_reduce(
                maxes[:, sc: sc + 1], chunks[sc][:, :n], axis=mybir.AxisListType.X,
                op=mybir.AluOpType.max,
            )
        negmax = st_pool.tile([16, 1], dt.float32, tag="negmax")
        nc.vector.tensor_reduce(
            negmax[:], maxes[:], axis=mybir.AxisListType.X,
            op=mybir.AluOpType.max, negate=True,
        )
        expv = sm_pool.tile([16, SP], dt.float32, tag="exp")
        nc.vector.memset(expv[:, S:], 0.0)
        sums = st_pool.tile([16, 5], dt.float32, tag="sums")
        for sc in range(5):
            n = 512 if sc < 4 else 1
            nc.scalar.activation(expv[:, sc * 512: sc * 512 + n], chunks[sc][:, :n],
                                 AF.Exp, bias=negmax[:], scale=1.0,
                                 accum_out=sums[:, sc: sc + 1])
        sumexp = st_pool.tile([16, 1], dt.float32, tag="sumexp")
        nc.vector.tensor_reduce(
            sumexp[:], sums[:], axis=mybir.AxisListType.X, op=mybir.AluOpType.add)
        recip = st_pool.tile([16, 1], dt.float32, tag="recip")
        nc.vector.reciprocal(recip[:], sumexp[:])
        attn = sm_pool.tile([16, SP], dt.bfloat16, tag="attn")
        nc.vector.tensor_scalar_mul(attn[:], expv[:], recip[:])

        # --- transpose attn -> attnT [128s, 16h] per s-tile ---
        atT = at_pool.tile([128, ST * 16], dt.bfloat16, tag="attnT")
        for st in range(ST):
            ps = ps_tr.tile([128, 16], dt.bfloat16, tag="tr")
            nc.tensor.transpose(ps[:], attn[:, st * 128:(st + 1) * 128], id_sb[:])
            nc.vector.tensor_copy(atT[:, st * 16:(st + 1) * 16], ps[:])
        attnT.append(atT)

        # --- xaT[d, h] = sum_s x[s, d] * attn[h, s] ---
        for d8 in range(8):
            ps = ps_xa.tile([128, 16], dt.float32, tag="xa")
            for st in range(ST):
                nc.tensor.matmul(
                    ps[:],
                    xn_sb[b][:, st * 1024 + d8 * 128: st * 1024 + (d8 + 1) * 128],
                    atT[:, st * 16:(st + 1) * 16],
                    start=(st == 0), stop=(st == ST - 1),
                )
            nc.vector.tensor_copy(
                xaT_sb[:, d8 * 32 + b: d8 * 32 + 32: 2], ps[:])

    # --- cls: per head-pair into partition halves ---
    for dp in range(8):
        ps = ps_sm.tile([128, BL], dt.float32, tag="tr", name=f"cls{dp}")
        for half, h in ((0, 2 * dp), (1, 2 * dp + 1)):
            outp = ps[:64, :] if half == 0 else ps[64:128, :]
            for d8 in range(8):
                nc.tensor.matmul(
                    outp,
                    wv_sb[:, d8 * 1024 + h * 64: d8 * 1024 + (h + 1) * 64],
                    xaT_sb[:, d8 * 32 + 2 * h: d8 * 32 + 2 * h + 2],
                    start=(d8 == 0), stop=(d8 == 7),
                    tile_position=(0, 64 * half),
                )
        nc.vector.tensor_copy(cls_sb[:, dp * 2: dp * 2 + 2], ps[:])

    # --- proj + bias ---
    for fb in range(8):
        ps = ps_sm.tile([128, BL], dt.float32, tag="tr", name=f"proj{fb}")
        for dp in range(8):
            nc.tensor.matmul(
                ps[:],
                wp_sb[:, dp * 1024 + fb * 128: dp * 1024 + (fb + 1) * 128],
                cls_sb[:, dp * 2: dp * 2 + 2],
                start=(dp == 0), stop=(dp == 7),
            )
        nc.vector.tensor_scalar_add(out_sb[:, fb * 2: fb * 2 + 2], ps[:],
                                    bp_sb[:, fb: fb + 1])

    nc.sync.dma_start(out_d, out_sb[:])


def _build():
    if "nc" in _cached:
        return _cached["nc"]
    from contextlib import ExitStack
    import concourse.tile as tile
    from concourse import bacc

    nc = bacc.Bacc("TRN2", target_bir_lowering=False, debug=False,
                   num_devices=NCORES)
    with tile.TileContext(nc) as tc:
        with ExitStack() as ctx:
            _kernel_body(ctx, tc)
    nc.compile()
    _cached["nc"] = nc
    return nc


def _host_prep(x, w_qkv, w_proj, b_proj):
    x = np.asarray(x, dtype=np.float32)
    w_qkv = np.asarray(w_qkv, dtype=np.float32)
    w_proj = np.asarray(w_proj, dtype=np.float32)
    b_proj = np.asarray(b_proj, dtype=np.float32)

    w_q, w_k = w_qkv[:D], w_qkv[D:2 * D]
    q0 = x[:, 0, :] @ w_q.T                                   # [B, D]
    wfold = np.einsum("bhe,hed->bhd", q0.reshape(B, H, E),
                      w_k.reshape(H, E, D)) * SCALE           # [B, H, D]
    wfT = np.ascontiguousarray(wfold.transpose(0, 2, 1))      # [B, D, H]

    xb = x.astype(BF16)                                       # [B, S, D]

    wv_dev = np.ascontiguousarray(w_qkv[2 * D:].T).astype(BF16)   # [d, he]
    wp_dev = np.ascontiguousarray(w_proj.T).astype(BF16)          # [d, f]
    bp_dev = np.ascontiguousarray(b_proj.reshape(8, 128).T)       # [p, fb]
    id_dev = np.eye(16, dtype=BF16)

    in_maps = []
    for c in range(NCORES):
        b0 = c * BL
        xs = xb[b0:b0 + BL]                                   # [BL, S, D]
        xn = np.zeros((BL, SP, D), dtype=BF16)
        xn[:, :S] = xs
        xt = np.ascontiguousarray(xs.transpose(0, 2, 1))      # [BL, D, S]
        wf_core = (wfT[b0:b0 + BL].reshape(BL, DT, 128, H)
                   .transpose(2, 0, 1, 3).reshape(128, BL * 128).astype(BF16))
        in_maps.append({
            "xn": xn.reshape(BL * SP, D),
            "xt": xt.reshape(BL * D, S),
            "wf": np.ascontiguousarray(wf_core),
            "wv": wv_dev,
            "wp": wp_dev,
            "bp": bp_dev,
            "ident": id_dev,
        })
    return x, in_maps


def _run(x, w_qkv, w_proj, b_proj, trace=False):
    from concourse import bass_utils

    nc = _build()
    x, in_maps = _host_prep(x, w_qkv, w_proj, b_proj)
    res = bass_utils.run_bass_kernel_spmd(
        nc, in_maps, core_ids=list(range(NCORES)), trace=trace)

    out = x.copy()
    for c in range(NCORES):
        dev = np.asarray(res.results[c]["out"], dtype=np.float32)  # [128, 16]
        cls = dev.reshape(128, 8, BL).transpose(2, 1, 0).reshape(BL, D)
        out[c * BL:(c + 1) * BL, 0, :] = cls
    return out, res


def kernel(x, w_qkv, w_proj, b_proj):
    out, _ = _run(x, w_qkv, w_proj, b_proj, trace=False)
    return out


# revision 6
# speedup vs baseline: 1.0649x; 1.0400x over previous
"""ClassAttention kernel for 8x TRN2 NeuronCores (Bass/Tile).

Problem (hardcoded): x[16, 2049, 1024], w_qkv[3072, 1024], w_proj[1024, 1024],
b_proj[1024].  Reference computes qkv projection, class-token attention
(only query position 0 attends), projection of the class token, and returns
concat([cls_tok, x[:, 1:]], axis=1).

Only output row 0 is computed; rows 1.. are x passthrough (done on host at
gather time, mirroring the reference's concatenate).

Algebraic restructure (exact same math, far fewer FLOPs):
    q0[b]        = x[b,0] @ Wq^T                       (host, tiny)
    wfold[b,h,:] = SCALE * q0[b,h,:] @ Wk_h            (host, tiny: fold q0 into Wk)
    logits[b,h,s]= sum_d x[b,s,d] * wfold[b,h,d]       (device matmul over d)
    attn         = softmax_s(logits)                    (device)
    xaT[b,d,h]   = sum_s x[b,s,d] * attn[b,h,s]        (device matmul over s)
    cls[b,he]    = sum_d xaT[b,d,h] * WvT[d,he]        (device, per-head blocks)
    out0[b,f]    = sum_d cls[b,d] * WpT[d,f] + bp[f]   (device)

Sharding: data-parallel over batch, 2 batch elements per core (8 cores).
x is shipped in bf16 in both natural [s,d] and transposed [d,s] layouts so
both contractions stream from SBUF with the contraction on the partition dim.
"""

import os
import numpy as np
import ml_dtypes

BF16 = ml_dtypes.bfloat16

B, S, D, H, E = 16, 2049, 1024, 16, 64
SCALE = E ** -0.5
NCORES = 8
BL = B // NCORES          # batches per core = 2
ST = 17                   # s-tiles of 128 (padded)
SP = ST * 128             # 2176 padded sequence
DT = 8                    # d-tiles of 128
NEG_BIG = -30000.0        # exp() underflows to exactly 0 in fp32

_cached = {}


def _kernel_body(ctx, tc):
    import concourse.bass as bass
    from concourse import mybir

    nc = tc.nc
    dt = mybir.dt
    AF = mybir.ActivationFunctionType

    xn_d = nc.dram_tensor("xn", (BL * SP, D), dt.bfloat16, kind="ExternalInput").ap()
    xt_d = nc.dram_tensor("xt", (BL * D, S), dt.bfloat16, kind="ExternalInput").ap()
    wf_d = nc.dram_tensor("wf", (128, BL * 128), dt.bfloat16, kind="ExternalInput").ap()
    wv_d = nc.dram_tensor("wv", (D, D), dt.bfloat16, kind="ExternalInput").ap()
    wp_d = nc.dram_tensor("wp", (D, D), dt.bfloat16, kind="ExternalInput").ap()
    bp_d = nc.dram_tensor("bp", (128, 8), dt.float32, kind="ExternalInput").ap()
    id_d = nc.dram_tensor("ident", (16, 16), dt.bfloat16, kind="ExternalInput").ap()
    out_d = nc.dram_tensor("out", (128, 16), dt.float32, kind="ExternalOutput").ap()

    cpool = ctx.enter_context(tc.tile_pool(name="const", bufs=1))
    xn_pool = ctx.enter_context(tc.tile_pool(name="xn", bufs=1))
    xt_pool = ctx.enter_context(tc.tile_pool(name="xt", bufs=3))
    w_pool = ctx.enter_context(tc.tile_pool(name="w", bufs=1))
    sm_pool = ctx.enter_context(tc.tile_pool(name="sm", bufs=1))
    st_pool = ctx.enter_context(tc.tile_pool(name="stats", bufs=2))
    at_pool = ctx.enter_context(tc.tile_pool(name="attnT", bufs=2))
    acc_pool = ctx.enter_context(tc.tile_pool(name="acc", bufs=1))

    ps_log = ctx.enter_context(tc.tile_pool(name="pslog", bufs=1, space="PSUM"))
    ps_tr = ctx.enter_context(tc.tile_pool(name="pstr", bufs=2, space="PSUM"))
    ps_xa = ctx.enter_context(tc.tile_pool(name="psxa", bufs=1, space="PSUM"))
    ps_sm = ps_tr

    # --- constants / weights ---
    wf_sb = cpool.tile([128, BL * 128], dt.bfloat16, tag="wf")
    nc.sync.dma_start(wf_sb[:], wf_d)
    id_sb = cpool.tile([16, 16], dt.bfloat16, tag="ident")
    nc.sync.dma_start(id_sb[:], id_d)
    bp_sb = cpool.tile([128, 8], dt.float32, tag="bp")
    nc.sync.dma_start(bp_sb[:], bp_d)

    # persistent x tiles (natural layout) + weights; all DMA on the sync
    # HWDGE queue in consumption-priority order (FIFO per queue):
    #   consts, xt_b0, xt_b1(h0), xn_b0, xt_b1(h1), wv, xn_b1, wp
    # so early compute is never starved and the last arrival (wp) has the
    # shortest downstream chain (proj only).
    xn_sb = [xn_pool.tile([128, ST * 1024], dt.bfloat16, tag=f"xn{b}",
                          name=f"xn{b}")
             for b in range(BL)]
    wv_sb = w_pool.tile([128, DT * 1024], dt.bfloat16, tag="wv")
    wp_sb = w_pool.tile([128, DT * 1024], dt.bfloat16, tag="wp")

    def load_xn(b, st0, st1):
        src = xn_d[b * SP + st0 * 128: b * SP + st1 * 128, :]
        nc.sync.dma_start(
            xn_sb[b][:, st0 * 1024: st1 * 1024]
            .rearrange("p (st d) -> p st d", st=st1 - st0),
            src.rearrange("(st p) d -> p st d", p=128),
        )

    def load_w(t, src):
        nc.sync.dma_start(
            t[:].rearrange("p (k c) -> p k c", k=DT),
            src.rearrange("(k p) c -> p k c", p=128),
        )

    def load_xt(b, hh):
        t = xt_pool.tile([128, 4 * S], dt.bfloat16, tag="xt", name=f"xt{b}_{hh}")
        r0 = b * D + hh * 512
        src = xt_d[r0:r0 + 512, :].rearrange("(k p) s -> p k s", p=128)
        nc.sync.dma_start(t[:].rearrange("p (k s) -> p k s", k=4), src)
        return t

    # persistent accumulators
    xaT_sb = acc_pool.tile([128, DT * 2 * H], dt.bfloat16, tag="xaT")  # col=dt*32+h*2+b
    cls_sb = acc_pool.tile([128, DT * BL], dt.bfloat16, tag="clsT")    # col=dtp*2+b
    out_sb = acc_pool.tile([128, 16], dt.float32, tag="out")           # col=fb*2+b

    # -- DMA program order (= sync-queue FIFO order) --
    xt_tiles = {}
    xt_tiles[(0, 0)] = load_xt(0, 0)
    xt_tiles[(0, 1)] = load_xt(0, 1)
    xt_tiles[(1, 0)] = load_xt(1, 0)
    load_xn(0, 0, 6)
    load_xn(0, 6, 12)
    load_xn(0, 12, 17)
    xt_tiles[(1, 1)] = load_xt(1, 1)
    load_w(wv_sb, wv_d)
    load_xn(1, 0, 6)
    load_xn(1, 6, 12)
    load_xn(1, 12, 17)
    load_w(wp_sb, wp_d)

    def emit_logits(b):
        halves = [xt_tiles[(b, 0)], xt_tiles[(b, 1)]]
        chunks = [ps_log.tile([16, 512], dt.float32, tag=f"c{sc}", name=f"c{sc}_{b}")
                  for sc in range(5)]
        for d8 in range(8):
            xtt = halves[d8 // 4]
            lhs = wf_sb[:, b * 128 + d8 * 16: b * 128 + (d8 + 1) * 16]
            base = (d8 % 4) * S
            for sc in range(5):
                n = 512 if sc < 4 else 1
                nc.tensor.matmul(
                    chunks[sc][:, :n], lhs, xtt[:, base + sc * 512: base + sc * 512 + n],
                    start=(d8 == 0), stop=(d8 == 7),
                )
        return chunks

    def emit_softmax(b, chunks):
        maxes = st_pool.tile([16, 5], dt.float32, tag="maxes", name=f"maxes{b}")
        for sc in range(5):
            n = 512 if sc < 4 else 1
            nc.vector.tensor_reduce(
                maxes[:, sc: sc + 1], chunks[sc][:, :n], axis=mybir.AxisListType.X,
                op=mybir.AluOpType.max,
            )
        negmax = st_pool.tile([16, 1], dt.float32, tag="negmax", name=f"negmax{b}")
        nc.vector.tensor_reduce(
            negmax[:], maxes[:], axis=mybir.AxisListType.X,
            op=mybir.AluOpType.max, negate=True,
        )
        expv = sm_pool.tile([16, SP], dt.float32, tag="exp", name=f"exp{b}")
        nc.vector.memset(expv[:, S:], 0.0)
        sums = st_pool.tile([16, 5], dt.float32, tag="sums", name=f"sums{b}")
        for sc in range(5):
            n = 512 if sc < 4 else 1
            nc.scalar.activation(expv[:, sc * 512: sc * 512 + n], chunks[sc][:, :n],
                                 AF.Exp, bias=negmax[:], scale=1.0,
                                 accum_out=sums[:, sc: sc + 1])
        sumexp = st_pool.tile([16, 1], dt.float32, tag="sumexp", name=f"sumexp{b}")
        nc.vector.tensor_reduce(
            sumexp[:], sums[:], axis=mybir.AxisListType.X, op=mybir.AluOpType.add)
        recip = st_pool.tile([16, 1], dt.float32, tag="recip", name=f"recip{b}")
        nc.vector.reciprocal(recip[:], sumexp[:])
        attn = sm_pool.tile([16, SP], dt.bfloat16, tag="attn", name=f"attn{b}")
        nc.vector.tensor_scalar_mul(attn[:], expv[:], recip[:])
        return attn

    def emit_transposes(b, attn):
        atT = at_pool.tile([128, ST * 16], dt.bfloat16, tag="attnT", name=f"atT{b}")
        for st in range(ST):
            ps = ps_tr.tile([128, 16], dt.bfloat16, tag="tr", name=f"tr{b}_{st}")
            nc.tensor.transpose(ps[:], attn[:, st * 128:(st + 1) * 128], id_sb[:])
            nc.vector.tensor_copy(atT[:, st * 16:(st + 1) * 16], ps[:])
        return atT

    def emit_xa_serial(b):
        # d8-outer, one accumulator: fine when xn[b] already resident
        for d8 in range(8):
            ps = ps_xa.tile([128, 16], dt.float32, tag="xa", name=f"xa{b}_{d8}")
            for st in range(ST):
                nc.tensor.matmul(
                    ps[:],
                    xn_sb[b][:, st * 1024 + d8 * 128: st * 1024 + (d8 + 1) * 128],
                    attnT[b][:, st * 16:(st + 1) * 16],
                    start=(st == 0), stop=(st == ST - 1),
                )
            nc.vector.tensor_copy(
                xaT_sb[:, d8 * 32 + b: d8 * 32 + 32: 2], ps[:])

    def emit_xa_wide(b):
        # st-outer with 8 parallel accumulators reusing freed pool slots:
        # only the final s-tiles trail the last xn chunk's arrival
        accs = [ps_log.tile([128, 16], dt.float32, tag=f"c{j}", name=f"xw{b}_{j}")
                for j in range(5)]
        accs.append(ps_xa.tile([128, 16], dt.float32, tag="xa", name=f"xw{b}_5"))
        accs += [ps_tr.tile([128, 16], dt.float32, tag="tr", name=f"xw{b}_{6 + j}")
                 for j in range(2)]
        for st in range(ST):
            for d8 in range(8):
                nc.tensor.matmul(
                    accs[d8][:],
                    xn_sb[b][:, st * 1024 + d8 * 128: st * 1024 + (d8 + 1) * 128],
                    attnT[b][:, st * 16:(st + 1) * 16],
                    start=(st == 0), stop=(st == ST - 1),
                )
        for d8 in range(8):
            nc.vector.tensor_copy(
                xaT_sb[:, d8 * 32 + b: d8 * 32 + 32: 2], accs[d8][:])

    # --- stage-interleaved emission: each engine's FIFO matches readiness ---
    attnT = {}
    ch0 = emit_logits(0)
    attn0 = emit_softmax(0, ch0)
    attnT[0] = emit_transposes(0, attn0)
    ch1 = emit_logits(1)
    attn1 = emit_softmax(1, ch1)
    emit_xa_serial(0)
    attnT[1] = emit_transposes(1, attn1)
    emit_xa_wide(1)

    # --- cls: per head-pair into partition halves ---
    for dp in range(8):
        ps = ps_sm.tile([128, BL], dt.float32, tag="tr", name=f"cls{dp}")
        for half, h in ((0, 2 * dp), (1, 2 * dp + 1)):
            outp = ps[:64, :] if half == 0 else ps[64:128, :]
            for d8 in range(8):
                nc.tensor.matmul(
                    outp,
                    wv_sb[:, d8 * 1024 + h * 64: d8 * 1024 + (h + 1) * 64],
                    xaT_sb[:, d8 * 32 + 2 * h: d8 * 32 + 2 * h + 2],
                    start=(d8 == 0), stop=(d8 == 7),
                    tile_position=(0, 64 * half),
                )
        nc.vector.tensor_copy(cls_sb[:, dp * 2: dp * 2 + 2], ps[:])

    # --- proj + bias ---
    for fb in range(8):
        ps = ps_sm.tile([128, BL], dt.float32, tag="tr", name=f"proj{fb}")
        for dp in range(8):
            nc.tensor.matmul(
                ps[:],
                wp_sb[:, dp * 1024 + fb * 128: dp * 1024 + (fb + 1) * 128],
                cls_sb[:, dp * 2: dp * 2 + 2],
                start=(dp == 0), stop=(dp == 7),
            )
        nc.vector.tensor_scalar_add(out_sb[:, fb * 2: fb * 2 + 2], ps[:],
                                    bp_sb[:, fb: fb + 1])

    nc.sync.dma_start(out_d, out_sb[:])


def _build():
    if "nc" in _cached:
        return _cached["nc"]
    from contextlib import ExitStack
    import concourse.tile as tile
    from concourse import bacc

    nc = bacc.Bacc("TRN2", target_bir_lowering=False, debug=False,
                   num_devices=NCORES)
    with tile.TileContext(nc) as tc:
        with ExitStack() as ctx:
            _kernel_body(ctx, tc)
    nc.compile()
    _cached["nc"] = nc
    return nc


def _host_prep(x, w_qkv, w_proj, b_proj):
    x = np.asarray(x, dtype=np.float32)
    w_qkv = np.asarray(w_qkv, dtype=np.float32)
    w_proj = np.asarray(w_proj, dtype=np.float32)
    b_proj = np.asarray(b_proj, dtype=np.float32)

    w_q, w_k = w_qkv[:D], w_qkv[D:2 * D]
    q0 = x[:, 0, :] @ w_q.T                                   # [B, D]
    wfold = np.einsum("bhe,hed->bhd", q0.reshape(B, H, E),
                      w_k.reshape(H, E, D)) * SCALE           # [B, H, D]
    wfT = np.ascontiguousarray(wfold.transpose(0, 2, 1))      # [B, D, H]

    xb = x.astype(BF16)                                       # [B, S, D]

    wv_dev = np.ascontiguousarray(w_qkv[2 * D:].T).astype(BF16)   # [d, he]
    wp_dev = np.ascontiguousarray(w_proj.T).astype(BF16)          # [d, f]
    bp_dev = np.ascontiguousarray(b_proj.reshape(8, 128).T)       # [p, fb]
    id_dev = np.eye(16, dtype=BF16)

    in_maps = []
    for c in range(NCORES):
        b0 = c * BL
        xs = xb[b0:b0 + BL]                                   # [BL, S, D]
        xn = np.zeros((BL, SP, D), dtype=BF16)
        xn[:, :S] = xs
        xt = np.ascontiguousarray(xs.transpose(0, 2, 1))      # [BL, D, S]
        wf_core = (wfT[b0:b0 + BL].reshape(BL, DT, 128, H)
                   .transpose(2, 0, 1, 3).reshape(128, BL * 128).astype(BF16))
        in_maps.append({
            "xn": xn.reshape(BL * SP, D),
            "xt": xt.reshape(BL * D, S),
            "wf": np.ascontiguousarray(wf_core),
            "wv": wv_dev,
            "wp": wp_dev,
            "bp": bp_dev,
            "ident": id_dev,
        })
    return x, in_maps


def _run(x, w_qkv, w_proj, b_proj, trace=False):
    from concourse import bass_utils

    nc = _build()
    x, in_maps = _host_prep(x, w_qkv, w_proj, b_proj)
    res = bass_utils.run_bass_kernel_spmd(
        nc, in_maps, core_ids=list(range(NCORES)), trace=trace)

    out = x.copy()
    for c in range(NCORES):
        dev = np.asarray(res.results[c]["out"], dtype=np.float32)  # [128, 16]
        cls = dev.reshape(128, 8, BL).transpose(2, 1, 0).reshape(BL, D)
        out[c * BL:(c + 1) * BL, 0, :] = cls
    return out, res


def kernel(x, w_qkv, w_proj, b_proj):
    out, _ = _run(x, w_qkv, w_proj, b_proj, trace=False)
    return out


# revision 7
# speedup vs baseline: 1.1849x; 1.1127x over previous
"""ClassAttention kernel for 8x TRN2 NeuronCores (Bass/Tile).

Problem (hardcoded): x[16, 2049, 1024], w_qkv[3072, 1024], w_proj[1024, 1024],
b_proj[1024].  Reference computes qkv projection, class-token attention
(only query position 0 attends), projection of the class token, and returns
concat([cls_tok, x[:, 1:]], axis=1).

Only output row 0 is computed; rows 1.. are x passthrough (done on host at
gather time, mirroring the reference's concatenate).

Algebraic restructure (exact same math, far fewer FLOPs):
    q0[b]        = x[b,0] @ Wq^T                       (host, tiny)
    wfold[b,h,:] = SCALE * q0[b,h,:] @ Wk_h            (host, tiny: fold q0 into Wk)
    logits[b,h,s]= sum_d x[b,s,d] * wfold[b,h,d]       (device matmul over d)
    attn         = softmax_s(logits)                    (device)
    xaT[b,d,h]   = sum_s x[b,s,d] * attn[b,h,s]        (device matmul over s)
    cls[b,he]    = sum_d xaT[b,d,h] * WvT[d,he]        (device, per-head blocks)
    out0[b,f]    = sum_d cls[b,d] * WpT[d,f] + bp[f]   (device)

Sharding: data-parallel over batch, 2 batch elements per core (8 cores).
x is shipped in bf16 in both natural [s,d] and transposed [d,s] layouts so
both contractions stream from SBUF with the contraction on the partition dim.
"""

import os
import numpy as np
import ml_dtypes

BF16 = ml_dtypes.bfloat16

B, S, D, H, E = 16, 2049, 1024, 16, 64
SCALE = E ** -0.5
NCORES = 8
BL = B // NCORES          # batches per core = 2
ST = 17                   # s-tiles of 128 (padded)
SP = ST * 128             # 2176 padded sequence
DT = 8                    # d-tiles of 128
NEG_BIG = -30000.0        # exp() underflows to exactly 0 in fp32

_cached = {}


def _kernel_body(ctx, tc):
    import concourse.bass as bass
    from concourse import mybir

    nc = tc.nc
    dt = mybir.dt
    AF = mybir.ActivationFunctionType

    xn_d = nc.dram_tensor("xn", (BL * SP, D), dt.bfloat16, kind="ExternalInput").ap()
    xt_d = nc.dram_tensor("xt", (BL * D, S), dt.bfloat16, kind="ExternalInput").ap()
    wf_d = nc.dram_tensor("wf", (128, BL * 128), dt.bfloat16, kind="ExternalInput").ap()
    wv_d = nc.dram_tensor("wv", (D, D), dt.bfloat16, kind="ExternalInput").ap()
    wp_d = nc.dram_tensor("wp", (D, D), dt.bfloat16, kind="ExternalInput").ap()
    bp_d = nc.dram_tensor("bp", (128, 8), dt.float32, kind="ExternalInput").ap()
    id_d = nc.dram_tensor("ident", (16, 16), dt.bfloat16, kind="ExternalInput").ap()
    out_d = nc.dram_tensor("out", (128, 16), dt.float32, kind="ExternalOutput").ap()

    cpool = ctx.enter_context(tc.tile_pool(name="const", bufs=1))
    xn_pool = ctx.enter_context(tc.tile_pool(name="xn", bufs=1))
    xt_pool = ctx.enter_context(tc.tile_pool(name="xt", bufs=4))
    w_pool = ctx.enter_context(tc.tile_pool(name="w", bufs=1))
    sm_pool = ctx.enter_context(tc.tile_pool(name="sm", bufs=1))
    st_pool = ctx.enter_context(tc.tile_pool(name="stats", bufs=2))
    at_pool = ctx.enter_context(tc.tile_pool(name="attnT", bufs=2))
    acc_pool = ctx.enter_context(tc.tile_pool(name="acc", bufs=1))

    ps_log = ctx.enter_context(tc.tile_pool(name="pslog", bufs=1, space="PSUM"))
    ps_tr = ctx.enter_context(tc.tile_pool(name="pstr", bufs=2, space="PSUM"))
    ps_xa = ctx.enter_context(tc.tile_pool(name="psxa", bufs=1, space="PSUM"))
    ps_sm = ps_tr

    # --- constants / weights ---
    wf_sb = cpool.tile([128, BL * 128], dt.bfloat16, tag="wf")
    nc.sync.dma_start(wf_sb[:], wf_d)
    id_sb = cpool.tile([16, 16], dt.bfloat16, tag="ident")
    nc.sync.dma_start(id_sb[:], id_d)
    bp_sb = cpool.tile([128, 8], dt.float32, tag="bp")
    nc.sync.dma_start(bp_sb[:], bp_d)

    # persistent x tiles (natural layout) + weights; all DMA on the sync
    # HWDGE queue in consumption-priority order (FIFO per queue):
    #   consts, xt_b0, xt_b1(h0), xn_b0, xt_b1(h1), wv, xn_b1, wp
    # so early compute is never starved and the last arrival (wp) has the
    # shortest downstream chain (proj only).
    xn_sb = [xn_pool.tile([128, ST * 1024], dt.bfloat16, tag=f"xn{b}",
                          name=f"xn{b}")
             for b in range(BL)]
    wv_sb = w_pool.tile([128, DT * 1024], dt.bfloat16, tag="wv")
    wp_sb = w_pool.tile([128, DT * 1024], dt.bfloat16, tag="wp")

    def load_xn(b, st0, st1):
        src = xn_d[b * SP + st0 * 128: b * SP + st1 * 128, :]
        nc.sync.dma_start(
            xn_sb[b][:, st0 * 1024: st1 * 1024]
            .rearrange("p (st d) -> p st d", st=st1 - st0),
            src.rearrange("(st p) d -> p st d", p=128),
        )

    def load_w(t, src):
        nc.sync.dma_start(
            t[:].rearrange("p (k c) -> p k c", k=DT),
            src.rearrange("(k p) c -> p k c", p=128),
        )

    def load_xt(b, hh):
        t = xt_pool.tile([128, 4 * S], dt.bfloat16, tag="xt", name=f"xt{b}_{hh}")
        r0 = b * D + hh * 512
        src = xt_d[r0:r0 + 512, :].rearrange("(k p) s -> p k s", p=128)
        nc.sync.dma_start(t[:].rearrange("p (k s) -> p k s", k=4), src)
        return t

    # persistent accumulators
    xaT_sb = acc_pool.tile([128, DT * 2 * H], dt.bfloat16, tag="xaT")  # col=dt*32+h*2+b
    cls_sb = acc_pool.tile([128, DT * BL], dt.bfloat16, tag="clsT")    # col=dtp*2+b
    out_sb = acc_pool.tile([128, 16], dt.float32, tag="out")           # col=fb*2+b

    # -- DMA program order (= sync-queue FIFO order) --
    xt_tiles = {}
    xt_tiles[(0, 0)] = load_xt(0, 0)
    xt_tiles[(0, 1)] = load_xt(0, 1)
    xt_tiles[(1, 0)] = load_xt(1, 0)
    xt_tiles[(1, 1)] = load_xt(1, 1)
    load_xn(0, 0, 6)
    load_xn(0, 6, 12)
    load_xn(0, 12, 17)
    load_w(wv_sb, wv_d)
    load_xn(1, 0, 6)
    load_xn(1, 6, 12)
    load_xn(1, 12, 17)
    load_w(wp_sb, wp_d)

    def emit_logits(b):
        halves = [xt_tiles[(b, 0)], xt_tiles[(b, 1)]]
        chunks = [ps_log.tile([16, 512], dt.float32, tag=f"c{sc}", name=f"c{sc}_{b}")
                  for sc in range(5)]
        for d8 in range(8):
            xtt = halves[d8 // 4]
            lhs = wf_sb[:, b * 128 + d8 * 16: b * 128 + (d8 + 1) * 16]
            base = (d8 % 4) * S
            for sc in range(5):
                n = 512 if sc < 4 else 1
                nc.tensor.matmul(
                    chunks[sc][:, :n], lhs, xtt[:, base + sc * 512: base + sc * 512 + n],
                    start=(d8 == 0), stop=(d8 == 7),
                )
        return chunks

    def emit_softmax_pre(b, chunks):
        maxes = st_pool.tile([16, 5], dt.float32, tag="maxes", name=f"maxes{b}")
        for sc in range(5):
            n = 512 if sc < 4 else 1
            nc.vector.tensor_reduce(
                maxes[:, sc: sc + 1], chunks[sc][:, :n], axis=mybir.AxisListType.X,
                op=mybir.AluOpType.max,
            )
        negmax = st_pool.tile([16, 1], dt.float32, tag="negmax", name=f"negmax{b}")
        nc.vector.tensor_reduce(
            negmax[:], maxes[:], axis=mybir.AxisListType.X,
            op=mybir.AluOpType.max, negate=True,
        )
        expv = sm_pool.tile([16, SP], dt.float32, tag="exp", name=f"exp{b}")
        nc.vector.memset(expv[:, S:], 0.0)
        sums = st_pool.tile([16, 5], dt.float32, tag="sums", name=f"sums{b}")
        for sc in range(5):
            n = 512 if sc < 4 else 1
            nc.scalar.activation(expv[:, sc * 512: sc * 512 + n], chunks[sc][:, :n],
                                 AF.Exp, bias=negmax[:], scale=1.0,
                                 accum_out=sums[:, sc: sc + 1])
        return expv, sums

    def emit_softmax_post(b, expv, sums):
        sumexp = st_pool.tile([16, 1], dt.float32, tag="sumexp", name=f"sumexp{b}")
        nc.vector.tensor_reduce(
            sumexp[:], sums[:], axis=mybir.AxisListType.X, op=mybir.AluOpType.add)
        recip = st_pool.tile([16, 1], dt.float32, tag="recip", name=f"recip{b}")
        nc.vector.reciprocal(recip[:], sumexp[:])
        attn = sm_pool.tile([16, SP], dt.bfloat16, tag="attn", name=f"attn{b}")
        nc.vector.tensor_scalar_mul(attn[:], expv[:], recip[:])
        return attn

    def emit_transposes(b, attn):
        atT = at_pool.tile([128, ST * 16], dt.bfloat16, tag="attnT", name=f"atT{b}")
        for st in range(ST):
            ps = ps_tr.tile([128, 16], dt.bfloat16, tag="tr", name=f"tr{b}_{st}")
            nc.tensor.transpose(ps[:], attn[:, st * 128:(st + 1) * 128], id_sb[:])
            nc.vector.tensor_copy(atT[:, st * 16:(st + 1) * 16], ps[:])
        return atT

    def emit_xa_serial(b):
        # d8-outer, one accumulator: fine when xn[b] already resident
        for d8 in range(8):
            ps = ps_xa.tile([128, 16], dt.float32, tag="xa", name=f"xa{b}_{d8}")
            for st in range(ST):
                nc.tensor.matmul(
                    ps[:],
                    xn_sb[b][:, st * 1024 + d8 * 128: st * 1024 + (d8 + 1) * 128],
                    attnT[b][:, st * 16:(st + 1) * 16],
                    start=(st == 0), stop=(st == ST - 1),
                )
            nc.vector.tensor_copy(
                xaT_sb[:, d8 * 32 + b: d8 * 32 + 32: 2], ps[:])

    def emit_xa_wide(b):
        # st-outer with 8 parallel accumulators reusing freed pool slots:
        # only the final s-tiles trail the last xn chunk's arrival
        accs = [ps_log.tile([128, 16], dt.float32, tag=f"c{j}", name=f"xw{b}_{j}")
                for j in range(5)]
        accs.append(ps_xa.tile([128, 16], dt.float32, tag="xa", name=f"xw{b}_5"))
        accs += [ps_tr.tile([128, 16], dt.float32, tag="tr", name=f"xw{b}_{6 + j}")
                 for j in range(2)]
        for st in range(ST):
            for d8 in range(8):
                nc.tensor.matmul(
                    accs[d8][:],
                    xn_sb[b][:, st * 1024 + d8 * 128: st * 1024 + (d8 + 1) * 128],
                    attnT[b][:, st * 16:(st + 1) * 16],
                    start=(st == 0), stop=(st == ST - 1),
                )
        for d8 in range(8):
            nc.vector.tensor_copy(
                xaT_sb[:, d8 * 32 + b: d8 * 32 + 32: 2], accs[d8][:])

    # --- stage-interleaved emission: each engine's FIFO matches readiness ---
    attnT = {}
    ch0 = emit_logits(0)
    e0, s0 = emit_softmax_pre(0, ch0)
    attn0 = emit_softmax_post(0, e0, s0)
    ch1 = emit_logits(1)
    e1, s1 = emit_softmax_pre(1, ch1)
    attnT[0] = emit_transposes(0, attn0)
    attn1 = emit_softmax_post(1, e1, s1)
    emit_xa_serial(0)
    attnT[1] = emit_transposes(1, attn1)
    emit_xa_wide(1)

    # --- cls: per head-pair into partition halves ---
    for dp in range(8):
        ps = ps_sm.tile([128, BL], dt.float32, tag="tr", name=f"cls{dp}")
        for half, h in ((0, 2 * dp), (1, 2 * dp + 1)):
            outp = ps[:64, :] if half == 0 else ps[64:128, :]
            for d8 in range(8):
                nc.tensor.matmul(
                    outp,
                    wv_sb[:, d8 * 1024 + h * 64: d8 * 1024 + (h + 1) * 64],
                    xaT_sb[:, d8 * 32 + 2 * h: d8 * 32 + 2 * h + 2],
                    start=(d8 == 0), stop=(d8 == 7),
                    tile_position=(0, 64 * half),
                )
        nc.vector.tensor_copy(cls_sb[:, dp * 2: dp * 2 + 2], ps[:])

    # --- proj + bias ---
    for fb in range(8):
        ps = ps_sm.tile([128, BL], dt.float32, tag="tr", name=f"proj{fb}")
        for dp in range(8):
            nc.tensor.matmul(
                ps[:],
                wp_sb[:, dp * 1024 + fb * 128: dp * 1024 + (fb + 1) * 128],
                cls_sb[:, dp * 2: dp * 2 + 2],
                start=(dp == 0), stop=(dp == 7),
            )
        nc.vector.tensor_scalar_add(out_sb[:, fb * 2: fb * 2 + 2], ps[:],
                                    bp_sb[:, fb: fb + 1])

    nc.sync.dma_start(out_d, out_sb[:])


def _build():
    if "nc" in _cached:
        return _cached["nc"]
    from contextlib import ExitStack
    import concourse.tile as tile
    from concourse import bacc

    nc = bacc.Bacc("TRN2", target_bir_lowering=False, debug=False,
                   num_devices=NCORES)
    with tile.TileContext(nc) as tc:
        with ExitStack() as ctx:
            _kernel_body(ctx, tc)
    nc.compile()
    _cached["nc"] = nc
    return nc


def _host_prep(x, w_qkv, w_proj, b_proj):
    x = np.asarray(x, dtype=np.float32)
    w_qkv = np.asarray(w_qkv, dtype=np.float32)
    w_proj = np.asarray(w_proj, dtype=np.float32)
    b_proj = np.asarray(b_proj, dtype=np.float32)

    w_q, w_k = w_qkv[:D], w_qkv[D:2 * D]
    q0 = x[:, 0, :] @ w_q.T                                   # [B, D]
    wfold = np.einsum("bhe,hed->bhd", q0.reshape(B, H, E),
                      w_k.reshape(H, E, D)) * SCALE           # [B, H, D]
    wfT = np.ascontiguousarray(wfold.transpose(0, 2, 1))      # [B, D, H]

    xb = x.astype(BF16)                                       # [B, S, D]

    wv_dev = np.ascontiguousarray(w_qkv[2 * D:].T).astype(BF16)   # [d, he]
    wp_dev = np.ascontiguousarray(w_proj.T).astype(BF16)          # [d, f]
    bp_dev = np.ascontiguousarray(b_proj.reshape(8, 128).T)       # [p, fb]
    id_dev = np.eye(16, dtype=BF16)

    in_maps = []
    for c in range(NCORES):
        b0 = c * BL
        xs = xb[b0:b0 + BL]                                   # [BL, S, D]
        xn = np.zeros((BL, SP, D), dtype=BF16)
        xn[:, :S] = xs
        xt = np.ascontiguousarray(xs.transpose(0, 2, 1))      # [BL, D, S]
        wf_core = (wfT[b0:b0 + BL].reshape(BL, DT, 128, H)
                   .transpose(2, 0, 1, 3).reshape(128, BL * 128).astype(BF16))
        in_maps.append({
            "xn": xn.reshape(BL * SP, D),
            "xt": xt.reshape(BL * D, S),
            "wf": np.ascontiguousarray(wf_core),
            "wv": wv_dev,
            "wp": wp_dev,
            "bp": bp_dev,
            "ident": id_dev,
        })
    return x, in_maps


def _run(x, w_qkv, w_proj, b_proj, trace=False):
    from concourse import bass_utils

    nc = _build()
    x, in_maps = _host_prep(x, w_qkv, w_proj, b_proj)
    res = bass_utils.run_bass_kernel_spmd(
        nc, in_maps, core_ids=list(range(NCORES)), trace=trace)

    out = x.copy()
    for c in range(NCORES):
        dev = np.asarray(res.results[c]["out"], dtype=np.float32)  # [128, 16]
        cls = dev.reshape(128, 8, BL).transpose(2, 1, 0).reshape(BL, D)
        out[c * BL:(c + 1) * BL, 0, :] = cls
    return out, res


def kernel(x, w_qkv, w_proj, b_proj):
    out, _ = _run(x, w_qkv, w_proj, b_proj, trace=False)
    return out


# revision 8
# speedup vs baseline: 1.2957x; 1.0935x over previous
"""ClassAttention kernel for 8x TRN2 NeuronCores (Bass/Tile).

Problem (hardcoded): x[16, 2049, 1024], w_qkv[3072, 1024], w_proj[1024, 1024],
b_proj[1024].  Reference computes qkv projection, class-token attention
(only query position 0 attends), projection of the class token, and returns
concat([cls_tok, x[:, 1:]], axis=1).

Only output row 0 is computed; rows 1.. are x passthrough (done on host at
gather time, mirroring the reference's concatenate).

Algebraic restructure (exact same math, far fewer FLOPs):
    q0[b]        = x[b,0] @ Wq^T                       (host, tiny)
    wfold[b,h,:] = SCALE * q0[b,h,:] @ Wk_h            (host, tiny: fold q0 into Wk)
    logits[b,h,s]= sum_d x[b,s,d] * wfold[b,h,d]       (device matmul over d)
    attn         = softmax_s(logits)                    (device)
    xaT[b,d,h]   = sum_s x[b,s,d] * attn[b,h,s]        (device matmul over s)
    cls[b,he]    = sum_d xaT[b,d,h] * WvT[d,he]        (device, per-head blocks)
    out0[b,f]    = sum_d cls[b,d] * WpT[d,f] + bp[f]   (device)

Sharding: data-parallel over batch, 2 batch elements per core (8 cores).
x is shipped in bf16 in both natural [s,d] and transposed [d,s] layouts so
both contractions stream from SBUF with the contraction on the partition dim.
"""

import os
import numpy as np
import ml_dtypes

BF16 = ml_dtypes.bfloat16
FP8 = ml_dtypes.float8_e3m4

# dtype knobs for the two big x streams (bfloat16 | float8e3)
XT_DTYPE = os.environ.get("K_XT_DTYPE", "float8e3")
XN_DTYPE = os.environ.get("K_XN_DTYPE", "bfloat16")
_NP_OF = {"bfloat16": BF16, "float8e3": FP8}

B, S, D, H, E = 16, 2049, 1024, 16, 64
SCALE = E ** -0.5
NCORES = 8
BL = B // NCORES          # batches per core = 2
ST = 17                   # s-tiles of 128 (padded)
SP = ST * 128             # 2176 padded sequence
DT = 8                    # d-tiles of 128
NEG_BIG = -30000.0        # exp() underflows to exactly 0 in fp32

_cached = {}


def _kernel_body(ctx, tc):
    import concourse.bass as bass
    from concourse import mybir

    nc = tc.nc
    dt = mybir.dt
    AF = mybir.ActivationFunctionType

    xt_dt = getattr(dt, XT_DTYPE)
    xn_dt = getattr(dt, XN_DTYPE)
    xn_d = nc.dram_tensor("xn", (BL * SP, D), xn_dt, kind="ExternalInput").ap()
    xt_d = nc.dram_tensor("xt", (BL * D, S), xt_dt, kind="ExternalInput").ap()
    wf_d = nc.dram_tensor("wf", (128, BL * 128), dt.bfloat16, kind="ExternalInput").ap()
    wv_d = nc.dram_tensor("wv", (D, D), dt.bfloat16, kind="ExternalInput").ap()
    wp_d = nc.dram_tensor("wp", (D, D), dt.bfloat16, kind="ExternalInput").ap()
    bp_d = nc.dram_tensor("bp", (128, 8), dt.float32, kind="ExternalInput").ap()
    id_d = nc.dram_tensor("ident", (16, 16), dt.bfloat16, kind="ExternalInput").ap()
    out_d = nc.dram_tensor("out", (128, 16), dt.float32, kind="ExternalOutput").ap()

    cpool = ctx.enter_context(tc.tile_pool(name="const", bufs=1))
    xn_pool = ctx.enter_context(tc.tile_pool(name="xn", bufs=1))
    xt_pool = ctx.enter_context(tc.tile_pool(name="xt", bufs=4))
    w_pool = ctx.enter_context(tc.tile_pool(name="w", bufs=1))
    sm_pool = ctx.enter_context(tc.tile_pool(name="sm", bufs=1))
    st_pool = ctx.enter_context(tc.tile_pool(name="stats", bufs=2))
    at_pool = ctx.enter_context(tc.tile_pool(name="attnT", bufs=2))
    acc_pool = ctx.enter_context(tc.tile_pool(name="acc", bufs=1))

    ps_log = ctx.enter_context(tc.tile_pool(name="pslog", bufs=1, space="PSUM"))
    ps_tr = ctx.enter_context(tc.tile_pool(name="pstr", bufs=2, space="PSUM"))
    ps_xa = ctx.enter_context(tc.tile_pool(name="psxa", bufs=1, space="PSUM"))
    ps_sm = ps_tr

    # --- constants / weights ---
    wf_sb = cpool.tile([128, BL * 128], dt.bfloat16, tag="wf")
    nc.sync.dma_start(wf_sb[:], wf_d)
    id_sb = cpool.tile([16, 16], dt.bfloat16, tag="ident")
    nc.sync.dma_start(id_sb[:], id_d)
    bp_sb = cpool.tile([128, 8], dt.float32, tag="bp")
    nc.sync.dma_start(bp_sb[:], bp_d)

    # persistent x tiles (natural layout) + weights; all DMA on the sync
    # HWDGE queue in consumption-priority order (FIFO per queue):
    #   consts, xt_b0, xt_b1(h0), xn_b0, xt_b1(h1), wv, xn_b1, wp
    # so early compute is never starved and the last arrival (wp) has the
    # shortest downstream chain (proj only).
    xn_sb = [xn_pool.tile([128, ST * 1024], xn_dt, tag=f"xn{b}",
                          name=f"xn{b}")
             for b in range(BL)]
    wv_sb = w_pool.tile([128, DT * 1024], dt.bfloat16, tag="wv")
    wp_sb = w_pool.tile([128, DT * 1024], dt.bfloat16, tag="wp")

    def load_xn(b, st0, st1):
        src = xn_d[b * SP + st0 * 128: b * SP + st1 * 128, :]
        nc.sync.dma_start(
            xn_sb[b][:, st0 * 1024: st1 * 1024]
            .rearrange("p (st d) -> p st d", st=st1 - st0),
            src.rearrange("(st p) d -> p st d", p=128),
        )

    def load_w(t, src):
        nc.sync.dma_start(
            t[:].rearrange("p (k c) -> p k c", k=DT),
            src.rearrange("(k p) c -> p k c", p=128),
        )

    def load_xt(b, hh):
        t = xt_pool.tile([128, 4 * S], xt_dt, tag="xt", name=f"xt{b}_{hh}")
        r0 = b * D + hh * 512
        src = xt_d[r0:r0 + 512, :].rearrange("(k p) s -> p k s", p=128)
        nc.sync.dma_start(t[:].rearrange("p (k s) -> p k s", k=4), src)
        return t

    # persistent accumulators
    xaT_sb = acc_pool.tile([128, DT * 2 * H], dt.bfloat16, tag="xaT")  # col=dt*32+h*2+b
    cls_sb = acc_pool.tile([128, DT * BL], dt.bfloat16, tag="clsT")    # col=dtp*2+b
    out_sb = acc_pool.tile([128, 16], dt.float32, tag="out")           # col=fb*2+b

    # -- DMA program order (= sync-queue FIFO order) --
    xt_tiles = {}
    xt_tiles[(0, 0)] = load_xt(0, 0)
    xt_tiles[(0, 1)] = load_xt(0, 1)
    xt_tiles[(1, 0)] = load_xt(1, 0)
    xt_tiles[(1, 1)] = load_xt(1, 1)
    load_xn(0, 0, 6)
    load_xn(0, 6, 12)
    load_xn(0, 12, 17)
    load_w(wv_sb, wv_d)
    load_xn(1, 0, 6)
    load_xn(1, 6, 12)
    load_xn(1, 12, 17)
    load_w(wp_sb, wp_d)

    def emit_logits(b):
        halves = [xt_tiles[(b, 0)], xt_tiles[(b, 1)]]
        chunks = [ps_log.tile([16, 512], dt.float32, tag=f"c{sc}", name=f"c{sc}_{b}")
                  for sc in range(5)]
        for d8 in range(8):
            xtt = halves[d8 // 4]
            lhs = wf_sb[:, b * 128 + d8 * 16: b * 128 + (d8 + 1) * 16]
            base = (d8 % 4) * S
            for sc in range(5):
                n = 512 if sc < 4 else 1
                nc.tensor.matmul(
                    chunks[sc][:, :n], lhs, xtt[:, base + sc * 512: base + sc * 512 + n],
                    start=(d8 == 0), stop=(d8 == 7),
                )
        return chunks

    def emit_softmax_pre(b, chunks):
        maxes = st_pool.tile([16, 5], dt.float32, tag="maxes", name=f"maxes{b}")
        for sc in range(5):
            n = 512 if sc < 4 else 1
            nc.vector.tensor_reduce(
                maxes[:, sc: sc + 1], chunks[sc][:, :n], axis=mybir.AxisListType.X,
                op=mybir.AluOpType.max,
            )
        negmax = st_pool.tile([16, 1], dt.float32, tag="negmax", name=f"negmax{b}")
        nc.vector.tensor_reduce(
            negmax[:], maxes[:], axis=mybir.AxisListType.X,
            op=mybir.AluOpType.max, negate=True,
        )
        expv = sm_pool.tile([16, SP], dt.float32, tag="exp", name=f"exp{b}")
        nc.vector.memset(expv[:, S:], 0.0)
        sums = st_pool.tile([16, 5], dt.float32, tag="sums", name=f"sums{b}")
        for sc in range(5):
            n = 512 if sc < 4 else 1
            nc.scalar.activation(expv[:, sc * 512: sc * 512 + n], chunks[sc][:, :n],
                                 AF.Exp, bias=negmax[:], scale=1.0,
                                 accum_out=sums[:, sc: sc + 1])
        return expv, sums

    def emit_softmax_post(b, expv, sums):
        sumexp = st_pool.tile([16, 1], dt.float32, tag="sumexp", name=f"sumexp{b}")
        nc.vector.tensor_reduce(
            sumexp[:], sums[:], axis=mybir.AxisListType.X, op=mybir.AluOpType.add)
        recip = st_pool.tile([16, 1], dt.float32, tag="recip", name=f"recip{b}")
        nc.vector.reciprocal(recip[:], sumexp[:])
        attn = sm_pool.tile([16, SP], dt.bfloat16, tag="attn", name=f"attn{b}")
        nc.vector.tensor_scalar_mul(attn[:], expv[:], recip[:])
        return attn

    def emit_transposes(b, attn):
        atT = at_pool.tile([128, ST * 16], dt.bfloat16, tag="attnT", name=f"atT{b}")
        for st in range(ST):
            ps = ps_tr.tile([128, 16], dt.bfloat16, tag="tr", name=f"tr{b}_{st}")
            nc.tensor.transpose(ps[:], attn[:, st * 128:(st + 1) * 128], id_sb[:])
            nc.vector.tensor_copy(atT[:, st * 16:(st + 1) * 16], ps[:])
        return atT

    def emit_xa_serial(b):
        # d8-outer, one accumulator: fine when xn[b] already resident
        for d8 in range(8):
            ps = ps_xa.tile([128, 16], dt.float32, tag="xa", name=f"xa{b}_{d8}")
            for st in range(ST):
                nc.tensor.matmul(
                    ps[:],
                    xn_sb[b][:, st * 1024 + d8 * 128: st * 1024 + (d8 + 1) * 128],
                    attnT[b][:, st * 16:(st + 1) * 16],
                    start=(st == 0), stop=(st == ST - 1),
                )
            nc.vector.tensor_copy(
                xaT_sb[:, d8 * 32 + b: d8 * 32 + 32: 2], ps[:])

    def emit_xa_wide(b):
        # st-outer with 8 parallel accumulators reusing freed pool slots:
        # only the final s-tiles trail the last xn chunk's arrival
        accs = [ps_log.tile([128, 16], dt.float32, tag=f"c{j}", name=f"xw{b}_{j}")
                for j in range(5)]
        accs.append(ps_xa.tile([128, 16], dt.float32, tag="xa", name=f"xw{b}_5"))
        accs += [ps_tr.tile([128, 16], dt.float32, tag="tr", name=f"xw{b}_{6 + j}")
                 for j in range(2)]
        for st in range(ST):
            for d8 in range(8):
                nc.tensor.matmul(
                    accs[d8][:],
                    xn_sb[b][:, st * 1024 + d8 * 128: st * 1024 + (d8 + 1) * 128],
                    attnT[b][:, st * 16:(st + 1) * 16],
                    start=(st == 0), stop=(st == ST - 1),
                )
        for d8 in range(8):
            nc.vector.tensor_copy(
                xaT_sb[:, d8 * 32 + b: d8 * 32 + 32: 2], accs[d8][:])

    # --- stage-interleaved emission: each engine's FIFO matches readiness ---
    attnT = {}
    ch0 = emit_logits(0)
    e0, s0 = emit_softmax_pre(0, ch0)
    attn0 = emit_softmax_post(0, e0, s0)
    ch1 = emit_logits(1)
    e1, s1 = emit_softmax_pre(1, ch1)
    attnT[0] = emit_transposes(0, attn0)
    attn1 = emit_softmax_post(1, e1, s1)
    emit_xa_serial(0)
    attnT[1] = emit_transposes(1, attn1)
    emit_xa_wide(1)

    # --- cls: per head-pair into partition halves ---
    for dp in range(8):
        ps = ps_sm.tile([128, BL], dt.float32, tag="tr", name=f"cls{dp}")
        for half, h in ((0, 2 * dp), (1, 2 * dp + 1)):
            outp = ps[:64, :] if half == 0 else ps[64:128, :]
            for d8 in range(8):
                nc.tensor.matmul(
                    outp,
                    wv_sb[:, d8 * 1024 + h * 64: d8 * 1024 + (h + 1) * 64],
                    xaT_sb[:, d8 * 32 + 2 * h: d8 * 32 + 2 * h + 2],
                    start=(d8 == 0), stop=(d8 == 7),
                    tile_position=(0, 64 * half),
                )
        nc.vector.tensor_copy(cls_sb[:, dp * 2: dp * 2 + 2], ps[:])

    # --- proj + bias ---
    for fb in range(8):
        ps = ps_sm.tile([128, BL], dt.float32, tag="tr", name=f"proj{fb}")
        for dp in range(8):
            nc.tensor.matmul(
                ps[:],
                wp_sb[:, dp * 1024 + fb * 128: dp * 1024 + (fb + 1) * 128],
                cls_sb[:, dp * 2: dp * 2 + 2],
                start=(dp == 0), stop=(dp == 7),
            )
        nc.vector.tensor_scalar_add(out_sb[:, fb * 2: fb * 2 + 2], ps[:],
                                    bp_sb[:, fb: fb + 1])

    nc.sync.dma_start(out_d, out_sb[:])


def _build():
    if "nc" in _cached:
        return _cached["nc"]
    from contextlib import ExitStack
    import concourse.tile as tile
    from concourse import bacc

    nc = bacc.Bacc("TRN2", target_bir_lowering=False, debug=False,
                   num_devices=NCORES)
    with tile.TileContext(nc) as tc:
        with ExitStack() as ctx:
            _kernel_body(ctx, tc)
    nc.compile()
    _cached["nc"] = nc
    return nc


def _host_prep(x, w_qkv, w_proj, b_proj):
    x = np.asarray(x, dtype=np.float32)
    w_qkv = np.asarray(w_qkv, dtype=np.float32)
    w_proj = np.asarray(w_proj, dtype=np.float32)
    b_proj = np.asarray(b_proj, dtype=np.float32)

    w_q, w_k = w_qkv[:D], w_qkv[D:2 * D]
    q0 = x[:, 0, :] @ w_q.T                                   # [B, D]
    wfold = np.einsum("bhe,hed->bhd", q0.reshape(B, H, E),
                      w_k.reshape(H, E, D)) * SCALE           # [B, H, D]
    wfT = np.ascontiguousarray(wfold.transpose(0, 2, 1))      # [B, D, H]

    xtnp = _NP_OF[XT_DTYPE]
    xnnp = _NP_OF[XN_DTYPE]
    xc = np.clip(x, -15.0, 15.0) if (xtnp is FP8 or xnnp is FP8) else x

    wv_dev = np.ascontiguousarray(w_qkv[2 * D:].T).astype(BF16)   # [d, he]
    wp_dev = np.ascontiguousarray(w_proj.T).astype(BF16)          # [d, f]
    bp_dev = np.ascontiguousarray(b_proj.reshape(8, 128).T)       # [p, fb]
    id_dev = np.eye(16, dtype=BF16)

    in_maps = []
    for c in range(NCORES):
        b0 = c * BL
        xn = np.zeros((BL, SP, D), dtype=xnnp)
        xn[:, :S] = (x if xnnp is not FP8 else xc)[b0:b0 + BL].astype(xnnp)
        xt = np.ascontiguousarray(
            (x if xtnp is not FP8 else xc)[b0:b0 + BL].transpose(0, 2, 1)).astype(xtnp)
        wf_core = (wfT[b0:b0 + BL].reshape(BL, DT, 128, H)
                   .transpose(2, 0, 1, 3).reshape(128, BL * 128).astype(BF16))
        in_maps.append({
            "xn": xn.reshape(BL * SP, D),
            "xt": xt.reshape(BL * D, S),
            "wf": np.ascontiguousarray(wf_core),
            "wv": wv_dev,
            "wp": wp_dev,
            "bp": bp_dev,
            "ident": id_dev,
        })
    return x, in_maps


def _run(x, w_qkv, w_proj, b_proj, trace=False):
    from concourse import bass_utils

    nc = _build()
    x, in_maps = _host_prep(x, w_qkv, w_proj, b_proj)
    res = bass_utils.run_bass_kernel_spmd(
        nc, in_maps, core_ids=list(range(NCORES)), trace=trace)

    out = x.copy()
    for c in range(NCORES):
        dev = np.asarray(res.results[c]["out"], dtype=np.float32)  # [128, 16]
        cls = dev.reshape(128, 8, BL).transpose(2, 1, 0).reshape(BL, D)
        out[c * BL:(c + 1) * BL, 0, :] = cls
    return out, res


def kernel(x, w_qkv, w_proj, b_proj):
    out, _ = _run(x, w_qkv, w_proj, b_proj, trace=False)
    return out
